# revision 16
# baseline (speedup 1.0000x reference)
"""BiMamba Trainium2 kernel (v2 — engine-rebalanced scan phase).

Sharding: each of the 8 cores owns a 256-channel slice of d_inner for BOTH
directions (fwd+rev share in_proj/out_proj, so the reversed direction's
in_proj output is just a flipped view of the forward one).  Per core:
  - in_proj:  xz[:, slice] = hidden @ W_in[slice].T          (PE)
  - conv+silu: fwd reads the padded x buffer normally, rev reads it through
    reversed APs (zero pad on both ends)                     (ACT + DVE)
  - x_proj:   partial x_dbl summed over cores via 2 AllReduces (one per
    direction, so the fwd scan starts while the rev AR is in flight)
  - dt_proj + softplus (bf16 stationary+moving)              (PE + ACT)
  - selective scan per (dir, ptile, n):
        da  = exp(A_n * dt)                                  (ACT)
        dbx = dtx * B_n                                      (DVE/Pool split)
        h   = tensor_tensor_scan(dA, dBx)                    (DVE only)
        hC  = h * C_n  full-width; pair-tree accumulate      (DVE/Pool split)
    The dbx/hC/add passes are load-balanced between DVE and GpSimd by a
    greedy cost model; scans can only run on DVE (ISA limitation).
  - gate: y = y_ssm * silu(z); y_rev stored time-flipped     (DVE)
  - out_proj: f+r fused into one 4-matmul PSUM accumulation per block;
    ReduceScatter pipelined in 4 column chunks               (PE + collective)
"""

import os
import sys

sys.path.insert(0, "/opt/trn_rl_repo")

import numpy as np
import ml_dtypes

# ---------------------------------------------------------------- constants
P = 128           # partitions
L = 2048          # sequence length
DM = 1024         # d_model
DI = 2048         # d_inner
NST = 16          # d_state
RK = 64           # dt_rank
KCONV = 4         # conv width
NCORES = 8
CH = DI // NCORES          # channels per core per direction = 256
NPT = CH // P              # channel ptiles per core = 2
FB = 512                   # matmul moving free chunk
NFB = L // FB              # 4
PAD = KCONV - 1            # causal pad = 3
NXP = RK + 2 * NST         # 96
OCHUNK = 512               # out_proj/RS column chunk
NOC = DM // OCHUNK         # 2

# measured per-[128,2048]-pass costs (us) for the DVE/Pool balancer
COST_DVE_SCAN = 5.0
COST_DVE_TT = 2.60
COST_POOL_TT = 999.0  # Pool co-running poisons SBUF arbitration; keep it idle


def build_program(num_cores=NCORES, enable_asserts=False, skip_scan=False):
    """Build the SPMD Bass program (same NEFF on every core)."""
    import concourse.bass as bass
    import concourse.mybir as mybir
    import concourse.tile as tile
    from concourse import bacc
    from contextlib import ExitStack

    dt = mybir.dt
    AF = mybir.ActivationFunctionType
    OP = mybir.AluOpType

    nc = bacc.Bacc(
        "TRN2",
        target_bir_lowering=False,
        debug=False,
        enable_asserts=enable_asserts,
        num_devices=num_cores,
    )

    # ------------------------------------------------------------- dram I/O
    hidden = nc.dram_tensor("hidden", [L, DM], dt.float32, kind="ExternalInput")
    w_inT = nc.dram_tensor("w_inT", [DM, 2 * CH], dt.bfloat16, kind="ExternalInput")
    w_outT = nc.dram_tensor("w_outT", [CH, DM], dt.bfloat16, kind="ExternalInput")
    w_xT = {}
    w_dtT = {}
    conv_w = {}
    conv_b = {}
    dt_b = {}
    A_in = {}
    D_in = {}
    for d in ("f", "r"):
        w_xT[d] = nc.dram_tensor(f"w_xT_{d}", [CH, NXP], dt.bfloat16,
                                 kind="ExternalInput")
        w_dtT[d] = nc.dram_tensor(f"w_dtT_{d}", [RK, CH], dt.bfloat16,
                                  kind="ExternalInput")
        conv_w[d] = nc.dram_tensor(f"conv_w_{d}", [CH, KCONV], dt.float32,
                                   kind="ExternalInput")
        conv_b[d] = nc.dram_tensor(f"conv_b_{d}", [CH, 1], dt.float32,
                                   kind="ExternalInput")
        dt_b[d] = nc.dram_tensor(f"dt_b_{d}", [CH, 1], dt.float32,
                                 kind="ExternalInput")
        A_in[d] = nc.dram_tensor(f"A_{d}", [CH, NST], dt.float32,
                                 kind="ExternalInput")
        D_in[d] = nc.dram_tensor(f"D_{d}", [CH, 1], dt.float32,
                                 kind="ExternalInput")
    ident = nc.dram_tensor("ident", [P, P], dt.float32, kind="ExternalInput")
    out = nc.dram_tensor("out", [L // num_cores, DM], dt.float32,
                         kind="ExternalOutput")

    NKB = DM // P  # 8
    NTT = L // P   # 16
    WPAD = L + 2 * PAD  # padded x width (zeros both ends for the rev conv)

    # greedy DVE/Pool balancer for tensor-tensor passes
    load = {"dve": 0.0, "pool": 0.0}

    def pick_tt(frac=1.0):
        """Pick engine for a tensor-tensor pass covering `frac` of [128,L]."""
        cd = load["dve"] + COST_DVE_TT * frac
        cp = load["pool"] + COST_POOL_TT * frac
        if cd <= cp:
            load["dve"] = cd
            return nc.vector
        load["pool"] = cp
        return nc.gpsimd

    def charge_dve(us):
        load["dve"] += us

    with tile.TileContext(nc) as tc:
        ctx = ExitStack()
        with ctx:
            dram = ctx.enter_context(tc.tile_pool(name="dram", bufs=1, space="DRAM"))
            consts = ctx.enter_context(tc.tile_pool(name="consts", bufs=1))
            psum_mm = ctx.enter_context(
                tc.tile_pool(name="psum_mm", bufs=3, space="PSUM"))

            # ---------------------------------------------------- constants
            ident_sb = consts.tile([P, P], dt.float32)
            nc.sync.dma_start(ident_sb[:], ident[:])
            conv_w_sb = {}
            conv_b_sb = {}
            dt_b_sb = {}
            A_sb = {}
            D_sb = {}
            for d in ("f", "r"):
                for pb in range(NPT):
                    ps = slice(pb * P, (pb + 1) * P)
                    for nm, store, src, shape in (
                        ("cw", conv_w_sb, conv_w, [P, KCONV]),
                        ("cb", conv_b_sb, conv_b, [P, 1]),
                        ("db", dt_b_sb, dt_b, [P, 1]),
                        ("A", A_sb, A_in, [P, NST]),
                        ("Dc", D_sb, D_in, [P, 1]),
                    ):
                        t = consts.tile(shape, dt.float32, name=f"{nm}{d}{pb}",
                                        tag=f"{nm}{d}{pb}")
                        nc.sync.dma_start(t[:], src[d][ps, :])
                        store[d, pb] = t
            w_dt_sb = {}
            for d in ("f", "r"):
                w_dt_sb[d] = consts.tile([RK, CH], dt.bfloat16, name=f"wdt{d}",
                                         tag=f"wdt{d}")
                nc.sync.dma_start(w_dt_sb[d][:], w_dtT[d][:])
            w_x_sb = {}
            for d in ("f", "r"):
                for pb in range(NPT):
                    t = consts.tile([P, NXP], dt.bfloat16, name=f"wx{d}{pb}",
                                    tag=f"wx{d}{pb}")
                    nc.sync.dma_start(t[:], w_xT[d][pb * P:(pb + 1) * P, :])
                    w_x_sb[d, pb] = t
            w_out_sb = []
            for pb in range(NPT):
                t = consts.tile([P, DM], dt.bfloat16, name=f"wo{pb}", tag=f"wo{pb}")
                nc.sync.dma_start(t[:], w_outT[pb * P:(pb + 1) * P, :])
                w_out_sb.append(t)

            # persistent activation buffers (gated-z, silu applied eagerly)
            gz_pool = ctx.enter_context(tc.tile_pool(name="gzp", bufs=1))
            gz = {}
            for d in ("f", "r"):
                for pb in range(NPT):
                    gz[d, pb] = gz_pool.tile([P, L], dt.bfloat16,
                                             name=f"gz{d}{pb}", tag=f"gz{d}{pb}")
            xc_pool = ctx.enter_context(tc.tile_pool(name="xcp", bufs=4))
            oev_pool = ctx.enter_context(tc.tile_pool(name="oevp", bufs=3))

            xdbl_part = dram.tile([2 * NXP, L], dt.bfloat16)
            xdbl_sum = {}
            for di, d in enumerate(("f", "r")):
                xdbl_sum[d] = dram.tile([NXP, L], dt.bfloat16,
                                        addr_space="Shared",
                                        name=f"xdbls{d}", tag=f"xdbls{d}")
            pout = [dram.tile([L, OCHUNK], dt.bfloat16, name=f"pout{oc}",
                              tag=f"pout{oc}") for oc in range(NOC)]
            pout_rs = [dram.tile([L // num_cores, OCHUNK], dt.bfloat16,
                                 name=f"poutrs{oc}", tag=f"poutrs{oc}")
                       for oc in range(NOC)]

            # stage-limited pools (freed once early phases are emitted)
            ctxB = ExitStack()
            xpad_pool = ctxB.enter_context(tc.tile_pool(name="xpadp", bufs=1))
            cacc_pool = ctxB.enter_context(tc.tile_pool(name="caccp", bufs=3))
            xev_pool = ctxB.enter_context(tc.tile_pool(name="xevp", bufs=2))
            ctxA = ExitStack()
            hT_pool = ctxA.enter_context(tc.tile_pool(name="hTp", bufs=1))
            hnat_pool = ctxA.enter_context(tc.tile_pool(name="hnatp", bufs=8))
            w_in_pool = ctxA.enter_context(tc.tile_pool(name="winp", bufs=1))

            # ------------------------------------------- stage 1: hT = hidden^T
            psum_tp = ctxA.enter_context(
                tc.tile_pool(name="psum_tp", bufs=3, space="PSUM"))
            hT = [hT_pool.tile([P, L], dt.bfloat16, name=f"hT{k}", tag=f"hT{k}")
                  for k in range(NKB)]
            for q in range(NTT // 4):
                hn = []
                for j in range(4):
                    t = hnat_pool.tile([P, DM], dt.float32, name="hnat", tag="hnat")
                    nc.sync.dma_start(
                        t[:], hidden[(q * 4 + j) * P:(q * 4 + j + 1) * P, :])
                    hn.append(t)
                for kb in range(NKB):
                    pt = psum_tp.tile([P, 4 * P], dt.float32, name="tp", tag="tp")
                    for j in range(4):
                        nc.tensor.transpose(
                            pt[:, j * P:(j + 1) * P],
                            hn[j][:, kb * P:(kb + 1) * P],
                            ident_sb[:],
                        )
                    nc.scalar.copy(hT[kb][:, q * 4 * P:(q + 1) * 4 * P], pt[:])

            # ------------------------------------------- stage 2: in_proj
            w_in_sb = [w_in_pool.tile([P, 2 * CH], dt.bfloat16, name=f"win{k}",
                                      tag=f"win{k}") for k in range(NKB)]
            for k in range(NKB):
                nc.sync.dma_start(w_in_sb[k][:], w_inT[k * P:(k + 1) * P, :])

            xpad = [xpad_pool.tile([P, WPAD], dt.bfloat16, name=f"xpad{pb}",
                                   tag=f"xpad{pb}") for pb in range(NPT)]
            for pb in range(NPT):
                nc.vector.memset(xpad[pb][:, 0:PAD], 0.0)
                nc.vector.memset(xpad[pb][:, PAD + L:WPAD], 0.0)

            def in_proj_mb(mb):
                for fb in range(NFB):
                    pm = psum_mm.tile([P, FB], dt.float32, name="mm", tag="mm")
                    for k in range(NKB):
                        nc.tensor.matmul(
                            pm[:],
                            w_in_sb[k][:, mb * P:(mb + 1) * P],
                            hT[k][:, fb * FB:(fb + 1) * FB],
                            start=(k == 0),
                            stop=(k == NKB - 1),
                        )
                    if mb < NPT:  # x half -> padded buffer (bf16)
                        nc.scalar.copy(
                            xpad[mb][:, PAD + fb * FB: PAD + (fb + 1) * FB], pm[:])
                    else:         # z half -> silu directly out of psum
                        pb = mb - NPT
                        nc.scalar.activation(
                            gz["f", pb][:, fb * FB:(fb + 1) * FB], pm[:], AF.Silu)
                        grev = gz["r", pb][:, ::-1]
                        nc.scalar.activation(
                            grev[:, fb * FB:(fb + 1) * FB], pm[:], AF.Silu)

            for mb in range(NPT):  # x halves first: conv+xproj+AR gate the scan
                in_proj_mb(mb)

            # -------------------------------- per-direction processing helpers
            xc = {}
            dt_sb = {}
            dtx = {}
            dxc = {}
            y = {}

            def conv_block(d):
                """causal depthwise conv + silu."""
                for pb in range(NPT):
                    cw = conv_w_sb[d, pb]
                    cb = conv_b_sb[d, pb]
                    if d == "f":
                        taps = [xpad[pb][:, k:k + L] for k in range(KCONV)]
                    else:
                        # reversed time: tap k reads xpad[2*PAD-k :][:L] reversed
                        taps = [xpad[pb][:, 2 * PAD - k: 2 * PAD - k + L][:, ::-1]
                                for k in range(KCONV)]
                    acc = cacc_pool.tile([P, L], dt.bfloat16, name="cacc", tag="cacc")
                    nc.scalar.activation(acc[:], taps[0], AF.Identity,
                                         bias=cb[:, 0:1], scale=cw[:, 0:1])
                    for k in range(1, KCONV):
                        acc2 = cacc_pool.tile([P, L], dt.bfloat16, name="cacc",
                                              tag="cacc")
                        nc.vector.scalar_tensor_tensor(
                            acc2[:], taps[k], cw[:, k:k + 1], acc[:],
                            OP.mult, OP.add)
                        charge_dve(COST_DVE_TT)
                        acc = acc2
                    t = xc_pool.tile([P, L], dt.bfloat16, name="xc", tag="xc")
                    nc.scalar.activation(t[:], acc[:], AF.Silu)
                    xc[d, pb] = t

            def xproj_block(d, di):
                for fb in range(NFB):
                    pm = psum_mm.tile([NXP, FB], dt.float32, name="mmx", tag="mm")
                    for pb in range(NPT):
                        nc.tensor.matmul(
                            pm[:],
                            w_x_sb[d, pb][:],
                            xc[d, pb][:, fb * FB:(fb + 1) * FB],
                            start=(pb == 0),
                            stop=(pb == NPT - 1),
                        )
                    xev = xev_pool.tile([NXP, FB], dt.bfloat16, name="xev", tag="xev")
                    nc.scalar.copy(xev[:], pm[:])
                    nc.sync.dma_start(
                        xdbl_part[di * NXP:(di + 1) * NXP, fb * FB:(fb + 1) * FB],
                        xev[:])

            xdbl16 = {}

            def dt_block(d):
                """Load x_dbl, cast B/C + dt-rows to bf16, dt_proj + softplus."""
                xdbl16[d] = xdbl16_pool.tile([RK, L], dt.bfloat16, name="xdbl16",
                                             tag="xdbl16")
                nc.sync.dma_start(xdbl16[d][:], xdbl_sum[d][0:RK, :])
                for pb in range(NPT):
                    t = dt_pool.tile([P, L], dt.bfloat16, name="dtt", tag="dtt")
                    for fb in range(NFB):
                        pm = psum_mm.tile([P, FB], dt.float32, name="mm", tag="mm")
                        nc.tensor.matmul(
                            pm[:],
                            w_dt_sb[d][:, pb * P:(pb + 1) * P],
                            xdbl16[d][:, fb * FB:(fb + 1) * FB],
                            start=True, stop=True)
                        et = etmp_pool.tile([P, FB], dt.float32, name="etmp",
                                            tag="etmp")
                        nc.scalar.activation(
                            et[:], pm[:], AF.Exp, bias=dt_b_sb[d, pb][:, 0:1])
                        nc.scalar.activation(
                            t[:, fb * FB:(fb + 1) * FB], et[:], AF.Ln, bias=1.0)
                    dt_sb[d, pb] = t
                    tx = dtx_pool.tile([P, L], dt.bfloat16, name="dtx", tag="dtx")
                    nc.vector.tensor_mul(tx[:], dt_sb[d, pb][:], xc[d, pb][:])
                    charge_dve(COST_DVE_TT * 1.3)
                    dtx[d, pb] = tx
                    # D * x on ACT (per-partition scale)
                    dc = dxc_pool.tile([P, L], dt.bfloat16, name="dxc", tag="dxc")
                    nc.scalar.activation(dc[:], xc[d, pb][:], AF.Identity,
                                         scale=D_sb[d, pb][:, 0:1])
                    dxc[d, pb] = dc

            def scan_block(d):
                """Selective scan for one direction; writes y[d, pb].

                y["r", pb] is stored in NATURAL (forward) time order so that
                out_proj can treat both directions symmetrically."""
                if skip_scan:
                    for pb in range(NPT):
                        yt = y_pool.tile([P, L], dt.bfloat16, name="y", tag="y")
                        nc.vector.tensor_mul(yt[:], dtx[d, pb][:], gz[d, pb][:])
                        y[d, pb] = yt
                    return
                # hc tiles accumulate on idle DMA engines: 4 chains of 4
                # states each (chain head holds the running sum), then 3
                # engine adds + D*x leaf + gate.
                chains = {pb: [] for pb in range(NPT)}

                for n in range(NST):
                    rb = xdbl_sum[d][RK + n:RK + n + 1, :]
                    rc = xdbl_sum[d][RK + NST + n:RK + NST + n + 1, :]
                    bb = bbc_pool.tile([P, L], dt.bfloat16, name="bbc", tag="bbc")
                    nc.sync.dma_start(
                        bb[:], bass.AP(rb.tensor, rb.offset, [[0, P], [1, L]]))
                    cbt = cbc_pool.tile([P, L], dt.bfloat16, name="cbc", tag="cbc")
                    nc.sync.dma_start(
                        cbt[:], bass.AP(rc.tensor, rc.offset, [[0, P], [1, L]]))
                    for pb in range(NPT):
                        da = da_pool.tile([P, L], dt.bfloat16, name="da", tag="da")
                        nc.scalar.activation(
                            da[:], dt_sb[d, pb][:], AF.Exp,
                            scale=A_sb[d, pb][:, n:n + 1])
                        dbx = dbx_pool.tile([P, L], dt.bfloat16, name="dbx",
                                            tag="dbx")
                        pick_tt().tensor_tensor(dbx[:], dtx[d, pb][:], bb[:],
                                                OP.mult)
                        h = h_pool.tile([P, L], dt.bfloat16, name="h", tag="h")
                        nc.vector.tensor_tensor_scan(
                            h[:], da[:], dbx[:], 0.0, OP.mult, OP.add)
                        charge_dve(COST_DVE_SCAN)
                        if n % 4 == 0:
                            # chain head: hc tile lives until the final merge
                            hc = hc_pool.tile([P, L], dt.bfloat16, name="hc",
                                              tag="hc")
                            pick_tt().tensor_tensor(hc[:], h[:], cbt[:], OP.mult)
                            chains[pb].append(hc)
                        else:
                            hc = hct_pool.tile([P, L], dt.bfloat16, name="hct",
                                               tag="hct")
                            pick_tt().tensor_tensor(hc[:], h[:], cbt[:], OP.mult)
                            # 4 column-chunk accum DMAs spread across queues
                            # so the read-modify-write latency stays ~8us
                            for ac in range(4):
                                s = slice(ac * (L // 4), (ac + 1) * (L // 4))
                                nc.gpsimd.dma_start(chains[pb][-1][:, s],
                                                    hc[:, s], accum_op=OP.add)

                for pb in range(NPT):
                    yt = y_pool.tile([P, L], dt.bfloat16, name="y", tag="y")
                    c0, c1, c2, c3 = chains[pb]
                    s0 = hct_pool.tile([P, L], dt.bfloat16, name="hct", tag="hct")
                    pick_tt().tensor_add(s0[:], c0[:], c1[:])
                    s1 = hct_pool.tile([P, L], dt.bfloat16, name="hct", tag="hct")
                    pick_tt().tensor_add(s1[:], c2[:], c3[:])
                    s2 = hct_pool.tile([P, L], dt.bfloat16, name="hct", tag="hct")
                    pick_tt().tensor_add(s2[:], s0[:], s1[:])
                    s3 = hct_pool.tile([P, L], dt.bfloat16, name="hct", tag="hct")
                    pick_tt().tensor_add(s3[:], s2[:], dxc[d, pb][:])
                    # gate; rev output written time-flipped to natural order
                    gzt = gz[d, pb]
                    if d == "f":
                        nc.vector.tensor_mul(yt[:], s3[:], gzt[:])
                    else:
                        nc.vector.tensor_mul(yt[:, ::-1], s3[:], gzt[:])
                    charge_dve(COST_DVE_TT)
                    y[d, pb] = yt

            for di, d in enumerate(("f", "r")):
                conv_block(d)
                xproj_block(d, di)
                # fire this direction's AllReduce as soon as its partials are
                # written; the fwd scan starts while the rev AR is in flight
                nc.gpsimd.collective_compute(
                    "AllReduce",
                    OP.add,
                    replica_groups=[list(range(num_cores))],
                    ins=[xdbl_part[di * NXP:(di + 1) * NXP, :].opt()],
                    outs=[xdbl_sum[d][:].opt()],
                )
            for mb in range(NPT, 2 * NPT):  # z halves (gate inputs, used late)
                in_proj_mb(mb)
            ctxA.close()
            ctxB.close()

            # scan-phase pools (allocated after the stage-1/2 pools freed)
            etmp_pool = ctx.enter_context(tc.tile_pool(name="etmpp", bufs=2))
            dt_pool = ctx.enter_context(tc.tile_pool(name="dtp", bufs=2))
            dtx_pool = ctx.enter_context(tc.tile_pool(name="dtxp", bufs=2))
            dxc_pool = ctx.enter_context(tc.tile_pool(name="dxcp", bufs=2))
            xdbl16_pool = ctx.enter_context(tc.tile_pool(name="xdbl16p", bufs=1))
            bbc_pool = ctx.enter_context(tc.tile_pool(name="bbcp", bufs=2))
            cbc_pool = ctx.enter_context(tc.tile_pool(name="cbcp", bufs=2))
            da_pool = ctx.enter_context(tc.tile_pool(name="dap", bufs=2))
            dbx_pool = ctx.enter_context(tc.tile_pool(name="dbxp", bufs=2))
            h_pool = ctx.enter_context(tc.tile_pool(name="hp", bufs=2))
            hc_pool = ctx.enter_context(tc.tile_pool(name="hcp", bufs=9))
            hct_pool = ctx.enter_context(tc.tile_pool(name="hctp", bufs=4))
            y_pool = ctx.enter_context(tc.tile_pool(name="yp", bufs=4))

            poutf_dram = dram.tile([L, DM], dt.bfloat16, name="poutf",
                                   tag="poutf")

            dt_block("f")
            scan_block("f")
            # fwd out_proj runs during the rev scan (PE/ACT/DMA are idle);
            # bf16 partials bounce through DRAM and merge into the rev
            # PSUM groups at the end
            for oc in range(NOC):
                cs = slice(oc * OCHUNK, (oc + 1) * OCHUNK)
                for tb in range(L // P):
                    ts = slice(tb * P, (tb + 1) * P)
                    pm = psum_mm.tile([P, OCHUNK], dt.float32, name="mmo",
                                      tag="mm")
                    for pb in range(NPT):
                        nc.tensor.matmul(
                            pm[:],
                            y["f", pb][:, ts],
                            w_out_sb[pb][:, cs],
                            start=(pb == 0),
                            stop=(pb == NPT - 1),
                        )
                    oevf = oev_pool.tile([P, OCHUNK], dt.bfloat16, name="oev",
                                         tag="oev")
                    nc.scalar.copy(oevf[:], pm[:])
                    nc.sync.dma_start(poutf_dram[ts, cs], oevf[:])

            dt_block("r")
            scan_block("r")
            for oc in range(NOC):
                cs = slice(oc * OCHUNK, (oc + 1) * OCHUNK)
                for tb in range(L // P):
                    ts = slice(tb * P, (tb + 1) * P)
                    pm = psum_mm.tile([P, OCHUNK], dt.float32, name="mmo",
                                      tag="mm")
                    for pb in range(NPT):
                        nc.tensor.matmul(
                            pm[:],
                            y["r", pb][:, ts],
                            w_out_sb[pb][:, cs],
                            start=(pb == 0),
                            stop=(pb == NPT - 1),
                        )
                    pfl = oev_pool.tile([P, OCHUNK], dt.bfloat16, name="oev",
                                        tag="oev")
                    nc.sync.dma_start(pfl[:], poutf_dram[ts, cs])
                    oev = oev_pool.tile([P, OCHUNK], dt.bfloat16, name="oev",
                                        tag="oev")
                    nc.vector.tensor_tensor(oev[:], pm[:], pfl[:], OP.add)
                    nc.sync.dma_start(pout[oc][ts, :], oev[:])
                nc.gpsimd.collective_compute(
                    "ReduceScatter",
                    OP.add,
                    replica_groups=[list(range(num_cores))],
                    ins=[pout[oc][:].opt()],
                    outs=[pout_rs[oc][:].opt()],
                )
                # gpsimd DMA casts bf16 -> fp32 on the way out; overlaps the
                # next oc chunk's matmuls/RS
                for hq in range(2):
                    rs2 = slice(hq * (L // num_cores // 2),
                                (hq + 1) * (L // num_cores // 2))
                    nc.gpsimd.dma_start(out[rs2, cs], pout_rs[oc][rs2, :])

    return nc


# ---------------------------------------------------------------- host side
def _make_in_maps(inputs):
    """Slice/transpose the full inputs into per-core input dicts."""
    h = np.ascontiguousarray(np.asarray(inputs["hidden_states"],
                                        dtype=np.float32).reshape(L, DM))
    w_in = np.asarray(inputs["in_proj_w"], dtype=np.float32)     # (2DI, DM)
    w_out = np.asarray(inputs["out_proj_w"], dtype=np.float32)   # (DM, DI)
    ident = np.eye(P, dtype=np.float32)

    in_maps = []
    for c in range(NCORES):
        sl = slice(c * CH, (c + 1) * CH)
        m = {"hidden": h, "ident": ident}
        w_slice = np.concatenate(
            [w_in[sl, :], w_in[DI + c * CH: DI + (c + 1) * CH, :]], axis=0)
        m["w_inT"] = np.ascontiguousarray(
            w_slice.T).astype(ml_dtypes.bfloat16)                 # (DM, 2CH)
        m["w_outT"] = np.ascontiguousarray(
            w_out[:, sl].T).astype(ml_dtypes.bfloat16)            # (CH, DM)
        for d, tag in (("f", "_f"), ("r", "_r")):
            w_x = np.asarray(inputs[f"x_proj_w{tag}"], dtype=np.float32)
            m[f"w_xT_{d}"] = np.ascontiguousarray(
                w_x[:, sl].T).astype(ml_dtypes.bfloat16)          # (CH, 96)
            w_dt = np.asarray(inputs[f"dt_proj_w{tag}"], dtype=np.float32)
            m[f"w_dtT_{d}"] = np.ascontiguousarray(
                w_dt[sl, :].T).astype(ml_dtypes.bfloat16)          # (RK, CH)
            m[f"conv_w_{d}"] = np.ascontiguousarray(
                np.asarray(inputs[f"conv_w{tag}"], dtype=np.float32)[sl, :])
            m[f"conv_b_{d}"] = np.ascontiguousarray(
                np.asarray(inputs[f"conv_b{tag}"], dtype=np.float32)[sl, None])
            m[f"dt_b_{d}"] = np.ascontiguousarray(
                np.asarray(inputs[f"dt_proj_b{tag}"], dtype=np.float32)[sl, None])
            m[f"A_{d}"] = np.ascontiguousarray(
                -np.exp(np.asarray(inputs[f"A_log{tag}"], dtype=np.float32)[sl, :]))
            m[f"D_{d}"] = np.ascontiguousarray(
                np.asarray(inputs[f"D{tag}"], dtype=np.float32)[sl, None])
        in_maps.append(m)
    return in_maps


_CACHED = {}


def _install_ntff_hook_shim():
    """The agent image's antenv lacks axon_hooks; provide it and register
    the ctypes-based NTFF profile hook from trn_agent_boot."""
    import types
    try:
        import antenv.axon_hooks  # noqa: F401
        return
    except ImportError:
        pass
    import antenv
    mod = types.ModuleType("antenv.axon_hooks")
    _state = {"h": None}
    mod.get_axon_ntff_profile_hook = lambda: _state["h"]
    mod.set_axon_ntff_profile_hook = lambda h: _state.__setitem__("h", h)
    sys.modules["antenv.axon_hooks"] = mod
    antenv.axon_hooks = mod
    try:
        from trn_agent_boot.trn_boot import _ntff_profile_via_ctypes
        hook = _ntff_profile_via_ctypes("/opt/axon/libaxon_pjrt.so")
        if hook is not None:
            mod.set_axon_ntff_profile_hook(hook)
    except Exception:
        pass


def _install_hook_err_capture():
    """Wrap the neuronx_cc hook so compile errors land in hook_err.log
    instead of being swallowed by the PJRT boundary."""
    import traceback
    import concourse.bass2jax as b2j
    if getattr(b2j, "_err_capture_installed", False):
        return
    orig = b2j.neuronx_cc_hook

    def wrapped(*a):
        try:
            return orig(*a)
        except Exception:
            with open("/tmp/hook_err.log", "w") as f:
                f.write(traceback.format_exc())
            raise

    b2j.neuronx_cc_hook = wrapped
    b2j._err_capture_installed = True


def kernel(**inputs):
    from concourse.bass_utils import run_bass_kernel_spmd

    _install_ntff_hook_shim()
    _install_hook_err_capture()

    if "nc" not in _CACHED:
        from concourse.bass_interp import get_hw_module
        nc = build_program(
            skip_scan=bool(int(os.environ.get("KERNEL_SKIP_SCAN", "0"))))
        nc.finalize()  # bacc: register allocation, library/ACT-table loads
        nc.m = get_hw_module(nc.m)  # strip sim-only callback instructions
        _CACHED["nc"] = nc
    nc = _CACHED["nc"]

    in_maps = _make_in_maps(inputs)
    res = run_bass_kernel_spmd(
        nc, in_maps, core_ids=list(range(NCORES)),
        trace=bool(int(os.environ.get("KERNEL_TRACE", "0"))),
    )
    _CACHED["last_result"] = res
    outs = [res.results[c]["out"] for c in range(NCORES)]
    full = np.concatenate(outs, axis=0).reshape(1, L, DM).astype(np.float32)
    return full


if __name__ == "__main__":
    nc = build_program()
    try:
        n = sum(len(bb.instructions) for bb in nc.main_func.blocks)
    except Exception:
        n = "?"
    print("build ok; instructions:", n)


# revision 17
# speedup vs baseline: 1.1577x; 1.1577x over previous
"""BiMamba Trainium2 kernel (v2 — engine-rebalanced scan phase).

Sharding: each of the 8 cores owns a 256-channel slice of d_inner for BOTH
directions (fwd+rev share in_proj/out_proj, so the reversed direction's
in_proj output is just a flipped view of the forward one).  Per core:
  - in_proj:  xz[:, slice] = hidden @ W_in[slice].T          (PE)
  - conv+silu: fwd reads the padded x buffer normally, rev reads it through
    reversed APs (zero pad on both ends)                     (ACT + DVE)
  - x_proj:   partial x_dbl summed over cores via 2 AllReduces (one per
    direction, so the fwd scan starts while the rev AR is in flight)
  - dt_proj + softplus (bf16 stationary+moving)              (PE + ACT)
  - selective scan per (dir, ptile, n):
        da  = exp(A_n * dt)                                  (ACT)
        dbx = dtx * B_n                                      (DVE/Pool split)
        h   = tensor_tensor_scan(dA, dBx)                    (DVE only)
        hC  = h * C_n  full-width; pair-tree accumulate      (DVE/Pool split)
    The dbx/hC/add passes are load-balanced between DVE and GpSimd by a
    greedy cost model; scans can only run on DVE (ISA limitation).
  - gate: y = y_ssm * silu(z); y_rev stored time-flipped     (DVE)
  - out_proj: f+r fused into one 4-matmul PSUM accumulation per block;
    ReduceScatter pipelined in 4 column chunks               (PE + collective)
"""

import os
import sys

sys.path.insert(0, "/opt/trn_rl_repo")

import numpy as np
import ml_dtypes

# ---------------------------------------------------------------- constants
P = 128           # partitions
L = 2048          # sequence length
DM = 1024         # d_model
DI = 2048         # d_inner
NST = 16          # d_state
RK = 64           # dt_rank
KCONV = 4         # conv width
NCORES = 8
CH = DI // NCORES          # channels per core per direction = 256
NPT = CH // P              # channel ptiles per core = 2
FB = 512                   # matmul moving free chunk
NFB = L // FB              # 4
PAD = KCONV - 1            # causal pad = 3
NXP = RK + 2 * NST         # 96
OCHUNK = 512               # out_proj/RS column chunk
NOC = DM // OCHUNK         # 2

# measured per-[128,2048]-pass costs (us) for the DVE/Pool balancer
COST_DVE_SCAN = 5.0
COST_DVE_TT = 2.60
COST_POOL_TT = 999.0  # Pool co-running poisons SBUF arbitration; keep it idle


def build_program(num_cores=NCORES, enable_asserts=False, skip_scan=False):
    """Build the SPMD Bass program (same NEFF on every core)."""
    import concourse.bass as bass
    import concourse.mybir as mybir
    import concourse.tile as tile
    from concourse import bacc
    from contextlib import ExitStack

    dt = mybir.dt
    AF = mybir.ActivationFunctionType
    OP = mybir.AluOpType

    nc = bacc.Bacc(
        "TRN2",
        target_bir_lowering=False,
        debug=False,
        enable_asserts=enable_asserts,
        num_devices=num_cores,
    )

    # ------------------------------------------------------------- dram I/O
    hidden = nc.dram_tensor("hidden", [L, DM], dt.float32, kind="ExternalInput")
    w_inT = nc.dram_tensor("w_inT", [DM, 2 * CH], dt.bfloat16, kind="ExternalInput")
    w_outT = nc.dram_tensor("w_outT", [CH, DM], dt.bfloat16, kind="ExternalInput")
    w_xT = {}
    w_dtT = {}
    conv_w = {}
    conv_b = {}
    dt_b = {}
    A_in = {}
    D_in = {}
    for d in ("f", "r"):
        w_xT[d] = nc.dram_tensor(f"w_xT_{d}", [CH, NXP], dt.bfloat16,
                                 kind="ExternalInput")
        w_dtT[d] = nc.dram_tensor(f"w_dtT_{d}", [RK, CH], dt.bfloat16,
                                  kind="ExternalInput")
        conv_w[d] = nc.dram_tensor(f"conv_w_{d}", [CH, KCONV], dt.float32,
                                   kind="ExternalInput")
        conv_b[d] = nc.dram_tensor(f"conv_b_{d}", [CH, 1], dt.float32,
                                   kind="ExternalInput")
        dt_b[d] = nc.dram_tensor(f"dt_b_{d}", [CH, 1], dt.float32,
                                 kind="ExternalInput")
        A_in[d] = nc.dram_tensor(f"A_{d}", [CH, NST], dt.float32,
                                 kind="ExternalInput")
        D_in[d] = nc.dram_tensor(f"D_{d}", [CH, 1], dt.float32,
                                 kind="ExternalInput")
    ident = nc.dram_tensor("ident", [P, P], dt.float32, kind="ExternalInput")
    out = nc.dram_tensor("out", [L // num_cores, DM], dt.float32,
                         kind="ExternalOutput")

    NKB = DM // P  # 8
    NTT = L // P   # 16
    WPAD = L + 2 * PAD  # padded x width (zeros both ends for the rev conv)

    # greedy DVE/Pool balancer for tensor-tensor passes
    load = {"dve": 0.0, "pool": 0.0}

    def pick_tt(frac=1.0):
        """Pick engine for a tensor-tensor pass covering `frac` of [128,L]."""
        cd = load["dve"] + COST_DVE_TT * frac
        cp = load["pool"] + COST_POOL_TT * frac
        if cd <= cp:
            load["dve"] = cd
            return nc.vector
        load["pool"] = cp
        return nc.gpsimd

    def charge_dve(us):
        load["dve"] += us

    with tile.TileContext(nc) as tc:
        ctx = ExitStack()
        with ctx:
            dram = ctx.enter_context(tc.tile_pool(name="dram", bufs=1, space="DRAM"))
            consts = ctx.enter_context(tc.tile_pool(name="consts", bufs=1))
            psum_mm = ctx.enter_context(
                tc.tile_pool(name="psum_mm", bufs=3, space="PSUM"))

            # ---------------------------------------------------- constants
            ident_sb = consts.tile([P, P], dt.float32)
            nc.sync.dma_start(ident_sb[:], ident[:])
            conv_w_sb = {}
            conv_b_sb = {}
            dt_b_sb = {}
            A_sb = {}
            D_sb = {}
            for d in ("f", "r"):
                for pb in range(NPT):
                    ps = slice(pb * P, (pb + 1) * P)
                    for nm, store, src, shape in (
                        ("cw", conv_w_sb, conv_w, [P, KCONV]),
                        ("cb", conv_b_sb, conv_b, [P, 1]),
                        ("db", dt_b_sb, dt_b, [P, 1]),
                        ("A", A_sb, A_in, [P, NST]),
                        ("Dc", D_sb, D_in, [P, 1]),
                    ):
                        t = consts.tile(shape, dt.float32, name=f"{nm}{d}{pb}",
                                        tag=f"{nm}{d}{pb}")
                        nc.sync.dma_start(t[:], src[d][ps, :])
                        store[d, pb] = t
            w_dt_sb = {}
            for d in ("f", "r"):
                w_dt_sb[d] = consts.tile([RK, CH], dt.bfloat16, name=f"wdt{d}",
                                         tag=f"wdt{d}")
                nc.sync.dma_start(w_dt_sb[d][:], w_dtT[d][:])
            w_x_sb = {}
            for d in ("f", "r"):
                for pb in range(NPT):
                    t = consts.tile([P, NXP], dt.bfloat16, name=f"wx{d}{pb}",
                                    tag=f"wx{d}{pb}")
                    nc.sync.dma_start(t[:], w_xT[d][pb * P:(pb + 1) * P, :])
                    w_x_sb[d, pb] = t
            w_out_sb = []
            for pb in range(NPT):
                t = consts.tile([P, DM], dt.bfloat16, name=f"wo{pb}", tag=f"wo{pb}")
                nc.sync.dma_start(t[:], w_outT[pb * P:(pb + 1) * P, :])
                w_out_sb.append(t)

            # persistent activation buffers (gated-z, silu applied eagerly)
            gz_pool = ctx.enter_context(tc.tile_pool(name="gzp", bufs=1))
            gz = {}
            for d in ("f", "r"):
                for pb in range(NPT):
                    gz[d, pb] = gz_pool.tile([P, L], dt.bfloat16,
                                             name=f"gz{d}{pb}", tag=f"gz{d}{pb}")
            xc_pool = ctx.enter_context(tc.tile_pool(name="xcp", bufs=4))
            oev_pool = ctx.enter_context(tc.tile_pool(name="oevp", bufs=3))

            xdbl_part = dram.tile([2 * NXP, L], dt.bfloat16)
            xdbl_sum = {}
            for di, d in enumerate(("f", "r")):
                xdbl_sum[d] = dram.tile([NXP, L], dt.bfloat16,
                                        addr_space="Shared",
                                        name=f"xdbls{d}", tag=f"xdbls{d}")
            pout = [dram.tile([L, OCHUNK], dt.bfloat16, name=f"pout{oc}",
                              tag=f"pout{oc}") for oc in range(NOC)]
            pout_rs = [dram.tile([L // num_cores, OCHUNK], dt.bfloat16,
                                 name=f"poutrs{oc}", tag=f"poutrs{oc}")
                       for oc in range(NOC)]

            # stage-limited pools (freed once early phases are emitted)
            ctxB = ExitStack()
            xpad_pool = ctxB.enter_context(tc.tile_pool(name="xpadp", bufs=1))
            cacc_pool = ctxB.enter_context(tc.tile_pool(name="caccp", bufs=3))
            xev_pool = ctxB.enter_context(tc.tile_pool(name="xevp", bufs=2))
            ctxA = ExitStack()
            hT_pool = ctxA.enter_context(tc.tile_pool(name="hTp", bufs=1))
            hnat_pool = ctxA.enter_context(tc.tile_pool(name="hnatp", bufs=8))
            w_in_pool = ctxA.enter_context(tc.tile_pool(name="winp", bufs=1))

            # ------------------------------------------- stage 1: hT = hidden^T
            psum_tp = ctxA.enter_context(
                tc.tile_pool(name="psum_tp", bufs=3, space="PSUM"))
            hT = [hT_pool.tile([P, L], dt.bfloat16, name=f"hT{k}", tag=f"hT{k}")
                  for k in range(NKB)]
            for q in range(NTT // 4):
                hn = []
                for j in range(4):
                    t = hnat_pool.tile([P, DM], dt.float32, name="hnat", tag="hnat")
                    nc.sync.dma_start(
                        t[:], hidden[(q * 4 + j) * P:(q * 4 + j + 1) * P, :])
                    hn.append(t)
                for kb in range(NKB):
                    pt = psum_tp.tile([P, 4 * P], dt.float32, name="tp", tag="tp")
                    for j in range(4):
                        nc.tensor.transpose(
                            pt[:, j * P:(j + 1) * P],
                            hn[j][:, kb * P:(kb + 1) * P],
                            ident_sb[:],
                        )
                    nc.scalar.copy(hT[kb][:, q * 4 * P:(q + 1) * 4 * P], pt[:])

            # ------------------------------------------- stage 2: in_proj
            w_in_sb = [w_in_pool.tile([P, 2 * CH], dt.bfloat16, name=f"win{k}",
                                      tag=f"win{k}") for k in range(NKB)]
            for k in range(NKB):
                nc.sync.dma_start(w_in_sb[k][:], w_inT[k * P:(k + 1) * P, :])

            xpad = [xpad_pool.tile([P, WPAD], dt.bfloat16, name=f"xpad{pb}",
                                   tag=f"xpad{pb}") for pb in range(NPT)]
            for pb in range(NPT):
                nc.vector.memset(xpad[pb][:, 0:PAD], 0.0)
                nc.vector.memset(xpad[pb][:, PAD + L:WPAD], 0.0)

            def in_proj_mb(mb):
                for fb in range(NFB):
                    pm = psum_mm.tile([P, FB], dt.float32, name="mm", tag="mm")
                    for k in range(NKB):
                        nc.tensor.matmul(
                            pm[:],
                            w_in_sb[k][:, mb * P:(mb + 1) * P],
                            hT[k][:, fb * FB:(fb + 1) * FB],
                            start=(k == 0),
                            stop=(k == NKB - 1),
                        )
                    if mb < NPT:  # x half -> padded buffer (bf16)
                        nc.scalar.copy(
                            xpad[mb][:, PAD + fb * FB: PAD + (fb + 1) * FB], pm[:])
                    else:         # z half -> silu directly out of psum
                        pb = mb - NPT
                        nc.scalar.activation(
                            gz["f", pb][:, fb * FB:(fb + 1) * FB], pm[:], AF.Silu)
                        grev = gz["r", pb][:, ::-1]
                        nc.scalar.activation(
                            grev[:, fb * FB:(fb + 1) * FB], pm[:], AF.Silu)

            for mb in range(NPT):  # x halves first: conv+xproj+AR gate the scan
                in_proj_mb(mb)

            # -------------------------------- per-direction processing helpers
            xc = {}
            dt_sb = {}
            dtx = {}
            dxc = {}
            y = {}

            def conv_block(d):
                """causal depthwise conv + silu."""
                for pb in range(NPT):
                    cw = conv_w_sb[d, pb]
                    cb = conv_b_sb[d, pb]
                    if d == "f":
                        taps = [xpad[pb][:, k:k + L] for k in range(KCONV)]
                    else:
                        # reversed time: tap k reads xpad[2*PAD-k :][:L] reversed
                        taps = [xpad[pb][:, 2 * PAD - k: 2 * PAD - k + L][:, ::-1]
                                for k in range(KCONV)]
                    acc = cacc_pool.tile([P, L], dt.bfloat16, name="cacc", tag="cacc")
                    nc.scalar.activation(acc[:], taps[0], AF.Identity,
                                         bias=cb[:, 0:1], scale=cw[:, 0:1])
                    for k in range(1, KCONV):
                        acc2 = cacc_pool.tile([P, L], dt.bfloat16, name="cacc",
                                              tag="cacc")
                        nc.vector.scalar_tensor_tensor(
                            acc2[:], taps[k], cw[:, k:k + 1], acc[:],
                            OP.mult, OP.add)
                        charge_dve(COST_DVE_TT)
                        acc = acc2
                    t = xc_pool.tile([P, L], dt.bfloat16, name="xc", tag="xc")
                    nc.scalar.activation(t[:], acc[:], AF.Silu)
                    xc[d, pb] = t

            def xproj_block(d, di):
                for fb in range(NFB):
                    pm = psum_mm.tile([NXP, FB], dt.float32, name="mmx", tag="mm")
                    for pb in range(NPT):
                        nc.tensor.matmul(
                            pm[:],
                            w_x_sb[d, pb][:],
                            xc[d, pb][:, fb * FB:(fb + 1) * FB],
                            start=(pb == 0),
                            stop=(pb == NPT - 1),
                        )
                    xev = xev_pool.tile([NXP, FB], dt.bfloat16, name="xev", tag="xev")
                    nc.scalar.copy(xev[:], pm[:])
                    nc.sync.dma_start(
                        xdbl_part[di * NXP:(di + 1) * NXP, fb * FB:(fb + 1) * FB],
                        xev[:])

            xdbl16 = {}

            def dt_block(d):
                """Load x_dbl, cast B/C + dt-rows to bf16, dt_proj + softplus."""
                xdbl16[d] = xdbl16_pool.tile([RK, L], dt.bfloat16, name="xdbl16",
                                             tag="xdbl16")
                nc.sync.dma_start(xdbl16[d][:], xdbl_sum[d][0:RK, :])
                for pb in range(NPT):
                    t = dt_pool.tile([P, L], dt.bfloat16, name="dtt", tag="dtt")
                    for fb in range(NFB):
                        pm = psum_mm.tile([P, FB], dt.float32, name="mm", tag="mm")
                        nc.tensor.matmul(
                            pm[:],
                            w_dt_sb[d][:, pb * P:(pb + 1) * P],
                            xdbl16[d][:, fb * FB:(fb + 1) * FB],
                            start=True, stop=True)
                        et = etmp_pool.tile([P, FB], dt.float32, name="etmp",
                                            tag="etmp")
                        nc.scalar.activation(
                            et[:], pm[:], AF.Exp, bias=dt_b_sb[d, pb][:, 0:1])
                        nc.scalar.activation(
                            t[:, fb * FB:(fb + 1) * FB], et[:], AF.Ln, bias=1.0)
                    dt_sb[d, pb] = t
                    tx = dtx_pool.tile([P, L], dt.bfloat16, name="dtx", tag="dtx")
                    nc.vector.tensor_mul(tx[:], dt_sb[d, pb][:], xc[d, pb][:])
                    charge_dve(COST_DVE_TT * 1.3)
                    dtx[d, pb] = tx
                    # D * x on ACT (per-partition scale)
                    dc = dxc_pool.tile([P, L], dt.bfloat16, name="dxc", tag="dxc")
                    nc.scalar.activation(dc[:], xc[d, pb][:], AF.Identity,
                                         scale=D_sb[d, pb][:, 0:1])
                    dxc[d, pb] = dc

            def scan_block(d):
                """Selective scan for one direction; writes y[d, pb].

                y["r", pb] is stored in NATURAL (forward) time order so that
                out_proj can treat both directions symmetrically."""
                if skip_scan:
                    for pb in range(NPT):
                        yt = y_pool.tile([P, L], dt.bfloat16, name="y", tag="y")
                        nc.vector.tensor_mul(yt[:], dtx[d, pb][:], gz[d, pb][:])
                        y[d, pb] = yt
                    return
                # hc tiles accumulate on idle DMA engines: 4 chains of 4
                # states each (chain head holds the running sum), then 3
                # engine adds + D*x leaf + gate.
                chains = {pb: [] for pb in range(NPT)}

                for n in range(NST):
                    rb = xdbl_sum[d][RK + n:RK + n + 1, :]
                    rc = xdbl_sum[d][RK + NST + n:RK + NST + n + 1, :]
                    bb = bbc_pool.tile([P, L], dt.bfloat16, name="bbc", tag="bbc")
                    nc.sync.dma_start(
                        bb[:], bass.AP(rb.tensor, rb.offset, [[0, P], [1, L]]))
                    cbt = cbc_pool.tile([P, L], dt.bfloat16, name="cbc", tag="cbc")
                    nc.sync.dma_start(
                        cbt[:], bass.AP(rc.tensor, rc.offset, [[0, P], [1, L]]))
                    for pb in range(NPT):
                        da = da_pool.tile([P, L], dt.bfloat16, name="da", tag="da")
                        nc.scalar.activation(
                            da[:], dt_sb[d, pb][:], AF.Exp,
                            scale=A_sb[d, pb][:, n:n + 1])
                        dbx = dbx_pool.tile([P, L], dt.bfloat16, name="dbx",
                                            tag="dbx")
                        pick_tt().tensor_tensor(dbx[:], dtx[d, pb][:], bb[:],
                                                OP.mult)
                        h = h_pool.tile([P, L], dt.bfloat16, name="h", tag="h")
                        nc.vector.tensor_tensor_scan(
                            h[:], da[:], dbx[:], 0.0, OP.mult, OP.add)
                        charge_dve(COST_DVE_SCAN)
                        if n % 4 == 0:
                            # chain head: hc tile lives until the final merge
                            hc = hc_pool.tile([P, L], dt.bfloat16, name="hc",
                                              tag="hc")
                            pick_tt().tensor_tensor(hc[:], h[:], cbt[:], OP.mult)
                            chains[pb].append(hc)
                        else:
                            hc = hct_pool.tile([P, L], dt.bfloat16, name="hct",
                                               tag="hct")
                            pick_tt().tensor_tensor(hc[:], h[:], cbt[:], OP.mult)
                            # 4 column-chunk accum DMAs spread across queues
                            # so the read-modify-write latency stays ~8us
                            for ac in range(4):
                                s = slice(ac * (L // 4), (ac + 1) * (L // 4))
                                nc.gpsimd.dma_start(chains[pb][-1][:, s],
                                                    hc[:, s], accum_op=OP.add)

                for pb in range(NPT):
                    yt = y_pool.tile([P, L], dt.bfloat16, name="y", tag="y")
                    c0, c1, c2, c3 = chains[pb]
                    s0 = hct_pool.tile([P, L], dt.bfloat16, name="hct", tag="hct")
                    pick_tt().tensor_add(s0[:], c0[:], c1[:])
                    s1 = hct_pool.tile([P, L], dt.bfloat16, name="hct", tag="hct")
                    pick_tt().tensor_add(s1[:], c2[:], c3[:])
                    s2 = hct_pool.tile([P, L], dt.bfloat16, name="hct", tag="hct")
                    pick_tt().tensor_add(s2[:], s0[:], s1[:])
                    s3 = hct_pool.tile([P, L], dt.bfloat16, name="hct", tag="hct")
                    pick_tt().tensor_add(s3[:], s2[:], dxc[d, pb][:])
                    # gate; rev output written time-flipped to natural order
                    gzt = gz[d, pb]
                    if d == "f":
                        nc.vector.tensor_mul(yt[:], s3[:], gzt[:])
                    else:
                        nc.vector.tensor_mul(yt[:, ::-1], s3[:], gzt[:])
                    charge_dve(COST_DVE_TT)
                    y[d, pb] = yt

            for di, d in enumerate(("f", "r")):
                conv_block(d)
                xproj_block(d, di)
                # fire this direction's AllReduce as soon as its partials are
                # written; the fwd scan starts while the rev AR is in flight
                nc.gpsimd.collective_compute(
                    "AllReduce",
                    OP.add,
                    replica_groups=[list(range(num_cores))],
                    ins=[xdbl_part[di * NXP:(di + 1) * NXP, :].opt()],
                    outs=[xdbl_sum[d][:].opt()],
                )
            for mb in range(NPT, 2 * NPT):  # z halves (gate inputs, used late)
                in_proj_mb(mb)
            ctxA.close()
            ctxB.close()

            # scan-phase pools (allocated after the stage-1/2 pools freed)
            etmp_pool = ctx.enter_context(tc.tile_pool(name="etmpp", bufs=2))
            dt_pool = ctx.enter_context(tc.tile_pool(name="dtp", bufs=2))
            dtx_pool = ctx.enter_context(tc.tile_pool(name="dtxp", bufs=2))
            dxc_pool = ctx.enter_context(tc.tile_pool(name="dxcp", bufs=2))
            xdbl16_pool = ctx.enter_context(tc.tile_pool(name="xdbl16p", bufs=1))
            bbc_pool = ctx.enter_context(tc.tile_pool(name="bbcp", bufs=2))
            cbc_pool = ctx.enter_context(tc.tile_pool(name="cbcp", bufs=2))
            da_pool = ctx.enter_context(tc.tile_pool(name="dap", bufs=2))
            dbx_pool = ctx.enter_context(tc.tile_pool(name="dbxp", bufs=2))
            h_pool = ctx.enter_context(tc.tile_pool(name="hp", bufs=2))
            hc_pool = ctx.enter_context(tc.tile_pool(name="hcp", bufs=9))
            hct_pool = ctx.enter_context(tc.tile_pool(name="hctp", bufs=4))
            y_pool = ctx.enter_context(tc.tile_pool(name="yp", bufs=4))

            for d in ("f", "r"):
                dt_block(d)
                scan_block(d)

            # out_proj: f and r fused into one 4-matmul PSUM accumulation
            for oc in range(NOC):
                cs = slice(oc * OCHUNK, (oc + 1) * OCHUNK)
                for tb in range(L // P):
                    ts = slice(tb * P, (tb + 1) * P)
                    pm = psum_mm.tile([P, OCHUNK], dt.float32, name="mmo",
                                      tag="mm")
                    k = 0
                    for dd in ("f", "r"):
                        for pb in range(NPT):
                            nc.tensor.matmul(
                                pm[:],
                                y[dd, pb][:, ts],
                                w_out_sb[pb][:, cs],
                                start=(k == 0),
                                stop=(k == 2 * NPT - 1),
                            )
                            k += 1
                    oev = oev_pool.tile([P, OCHUNK], dt.bfloat16, name="oev",
                                        tag="oev")
                    nc.scalar.copy(oev[:], pm[:])
                    nc.sync.dma_start(pout[oc][ts, :], oev[:])
                nc.gpsimd.collective_compute(
                    "ReduceScatter",
                    OP.add,
                    replica_groups=[list(range(num_cores))],
                    ins=[pout[oc][:].opt()],
                    outs=[pout_rs[oc][:].opt()],
                )
                # gpsimd DMA casts bf16 -> fp32 on the way out; overlaps the
                # next oc chunk's matmuls/RS
                for hq in range(2):
                    rs2 = slice(hq * (L // num_cores // 2),
                                (hq + 1) * (L // num_cores // 2))
                    nc.gpsimd.dma_start(out[rs2, cs], pout_rs[oc][rs2, :])

    return nc


# ---------------------------------------------------------------- host side
def _make_in_maps(inputs):
    """Slice/transpose the full inputs into per-core input dicts."""
    h = np.ascontiguousarray(np.asarray(inputs["hidden_states"],
                                        dtype=np.float32).reshape(L, DM))
    w_in = np.asarray(inputs["in_proj_w"], dtype=np.float32)     # (2DI, DM)
    w_out = np.asarray(inputs["out_proj_w"], dtype=np.float32)   # (DM, DI)
    ident = np.eye(P, dtype=np.float32)

    in_maps = []
    for c in range(NCORES):
        sl = slice(c * CH, (c + 1) * CH)
        m = {"hidden": h, "ident": ident}
        w_slice = np.concatenate(
            [w_in[sl, :], w_in[DI + c * CH: DI + (c + 1) * CH, :]], axis=0)
        m["w_inT"] = np.ascontiguousarray(
            w_slice.T).astype(ml_dtypes.bfloat16)                 # (DM, 2CH)
        m["w_outT"] = np.ascontiguousarray(
            w_out[:, sl].T).astype(ml_dtypes.bfloat16)            # (CH, DM)
        for d, tag in (("f", "_f"), ("r", "_r")):
            w_x = np.asarray(inputs[f"x_proj_w{tag}"], dtype=np.float32)
            m[f"w_xT_{d}"] = np.ascontiguousarray(
                w_x[:, sl].T).astype(ml_dtypes.bfloat16)          # (CH, 96)
            w_dt = np.asarray(inputs[f"dt_proj_w{tag}"], dtype=np.float32)
            m[f"w_dtT_{d}"] = np.ascontiguousarray(
                w_dt[sl, :].T).astype(ml_dtypes.bfloat16)          # (RK, CH)
            m[f"conv_w_{d}"] = np.ascontiguousarray(
                np.asarray(inputs[f"conv_w{tag}"], dtype=np.float32)[sl, :])
            m[f"conv_b_{d}"] = np.ascontiguousarray(
                np.asarray(inputs[f"conv_b{tag}"], dtype=np.float32)[sl, None])
            m[f"dt_b_{d}"] = np.ascontiguousarray(
                np.asarray(inputs[f"dt_proj_b{tag}"], dtype=np.float32)[sl, None])
            m[f"A_{d}"] = np.ascontiguousarray(
                -np.exp(np.asarray(inputs[f"A_log{tag}"], dtype=np.float32)[sl, :]))
            m[f"D_{d}"] = np.ascontiguousarray(
                np.asarray(inputs[f"D{tag}"], dtype=np.float32)[sl, None])
        in_maps.append(m)
    return in_maps


_CACHED = {}


def _install_ntff_hook_shim():
    """The agent image's antenv lacks axon_hooks; provide it and register
    the ctypes-based NTFF profile hook from trn_agent_boot."""
    import types
    try:
        import antenv.axon_hooks  # noqa: F401
        return
    except ImportError:
        pass
    import antenv
    mod = types.ModuleType("antenv.axon_hooks")
    _state = {"h": None}
    mod.get_axon_ntff_profile_hook = lambda: _state["h"]
    mod.set_axon_ntff_profile_hook = lambda h: _state.__setitem__("h", h)
    sys.modules["antenv.axon_hooks"] = mod
    antenv.axon_hooks = mod
    try:
        from trn_agent_boot.trn_boot import _ntff_profile_via_ctypes
        hook = _ntff_profile_via_ctypes("/opt/axon/libaxon_pjrt.so")
        if hook is not None:
            mod.set_axon_ntff_profile_hook(hook)
    except Exception:
        pass


def _install_hook_err_capture():
    """Wrap the neuronx_cc hook so compile errors land in hook_err.log
    instead of being swallowed by the PJRT boundary."""
    import traceback
    import concourse.bass2jax as b2j
    if getattr(b2j, "_err_capture_installed", False):
        return
    orig = b2j.neuronx_cc_hook

    def wrapped(*a):
        try:
            return orig(*a)
        except Exception:
            with open("/tmp/hook_err.log", "w") as f:
                f.write(traceback.format_exc())
            raise

    b2j.neuronx_cc_hook = wrapped
    b2j._err_capture_installed = True


def kernel(**inputs):
    from concourse.bass_utils import run_bass_kernel_spmd

    _install_ntff_hook_shim()
    _install_hook_err_capture()

    if "nc" not in _CACHED:
        from concourse.bass_interp import get_hw_module
        nc = build_program(
            skip_scan=bool(int(os.environ.get("KERNEL_SKIP_SCAN", "0"))))
        nc.finalize()  # bacc: register allocation, library/ACT-table loads
        nc.m = get_hw_module(nc.m)  # strip sim-only callback instructions
        _CACHED["nc"] = nc
    nc = _CACHED["nc"]

    in_maps = _make_in_maps(inputs)
    res = run_bass_kernel_spmd(
        nc, in_maps, core_ids=list(range(NCORES)),
        trace=bool(int(os.environ.get("KERNEL_TRACE", "0"))),
    )
    _CACHED["last_result"] = res
    outs = [res.results[c]["out"] for c in range(NCORES)]
    full = np.concatenate(outs, axis=0).reshape(1, L, DM).astype(np.float32)
    return full


if __name__ == "__main__":
    nc = build_program()
    try:
        n = sum(len(bb.instructions) for bb in nc.main_func.blocks)
    except Exception:
        n = "?"
    print("build ok; instructions:", n)
